# revision 28
# baseline (speedup 1.0000x reference)
"""Trainium2 Bass kernel: cubic B-spline upsampling x2 of a (2,3,96,96,96) volume.

Math: the reference op (recursive IIR prefilter along each spatial axis, then
an 8-tap stride-2 transposed conv along each axis) is linear and separable.
The whole per-axis operator is a dense 192x96 matrix M (built exactly on the
host in float64).  out = M (x) M (x) M applied along z, y, x.  M's prefilter
factor decays as pole^d (pole=-0.268), so a 48-row z'-slice of M only needs a
48-wide window of input z columns (truncation rel err ~1e-8, far below bf16).

Device strategy (8 NeuronCores, SPMD, no collectives):
  24 tasks = 6 (b,c) volumes x 4 z'-slices of 48 rows; 3 tasks per core.
  Per task, three matmul stages in "data-stationary" form (stationary operand
  = data tile, moving operand = spline matrix), chosen so NO transposes are
  needed:
    A: per x  (96 mms): lhsT = vol[z-win48, x, y-pad128] rhs=MzT(48,48) -> (y, z')
    B: per z' (48 mms): lhsT = L1[y96, z', x-pad128]     rhs=MT (96,192) -> (x, y')
    C: per 128-chunk of (z'y') (72 mms): lhsT = L2f[x96, c] rhs=MT -> (chunk, x')
  HW timing model (measured): back-to-back self-loading matmuls pipeline so a
  pair costs max(K_ldweights_rows, N_moving + ~18ns) cycles at 2.4 GHz; the
  banded K=48 stage A and K=96 B/C keep every stage moving-bound.  PSUM
  evacuation (the other big cost; only DVE and Activation can read PSUM) is
  alternated between those two engines in 4-PSUM-bank (1536 elem/partition)
  copies from a single ping-pong PSUM pool.
  Output is written partition-major ((z'y')%128 on partitions) so each
  out-DMA moves 4.6KB+ contiguous per partition; the host undoes the
  permutation.  Compute in bf16 (PSUM accumulates fp32); output bf16,
  upcast on host (rel err ~5.3e-3 total vs the reference).
"""

import math
import os
import sys

import numpy as np

for _p in ("/opt/trn_rl_repo",):
    if _p not in sys.path and os.path.isdir(_p):
        sys.path.insert(0, _p)

import ml_dtypes  # noqa: E402

BF16 = ml_dtypes.bfloat16

POLE = math.sqrt(3.0) - 2.0
GAIN = (1.0 - POLE) * (1.0 - 1.0 / POLE)  # 6.0
N = 96
F = 2
NOUT = N * F  # 192
NTASK_PER_CORE = 3
NCORES = 8
ZSLICE = NOUT // 4  # 48
ZWIN = 64  # banded z-window per 48-row z'-slice (32-aligned for memset)
WINS = (0, 8, 24, 32)  # window start per slice index


def _cubic(t):
    a = np.abs(t)
    out = (2.0 / 3.0 + (0.5 * a - 1.0) * a**2) * (a < 1)
    out = out + (-((a - 2.0) ** 3) / 6.0) * ((a >= 1) & (a < 2))
    return out


def _prefilter_mat(n):
    """96x96 matrix of the causal+anticausal cubic-spline prefilter (float64)."""
    p = POLE
    xm = np.eye(n, dtype=np.float64) * GAIN
    i = np.arange(n)
    pows = p**i + p ** (2 * n - 1 - i)
    c = np.zeros((n, n), dtype=np.float64)
    c[0] = (pows @ xm) * (p / (1.0 - p ** (2 * n))) + xm[0]
    for k in range(1, n):
        c[k] = xm[k] + p * c[k - 1]
    out = np.zeros((n, n), dtype=np.float64)
    out[n - 1] = c[n - 1] * (p / (p - 1.0))
    for k in range(n - 2, -1, -1):
        out[k] = p * (out[k + 1] - c[k])
    return out


def _upsample_mat(n, f=F):
    """2n x n matrix of the edge-padded stride-2 transposed conv (float64)."""
    k = 4 * f  # f even -> is_odd == 0
    start = 1.0 / (2 * f) - 2.0
    pts = np.arange(k, dtype=np.float64) * (1.0 / f) + start
    ker = _cubic(pts)
    npad = n + 4
    U = np.zeros((f * n, npad), dtype=np.float64)
    for o in range(f * n):
        for i in range(npad):
            s = o + (k - 1) - f * i
            if 0 <= s < k:
                U[o, i] += ker[s]
    Uc = np.zeros((f * n, n), dtype=np.float64)
    for i in range(npad):
        j = min(max(i - 2, 0), n - 1)
        Uc[:, j] += U[:, i]
    return Uc


def build_M():
    """Exact 192x96 per-axis operator (float64)."""
    return _upsample_mat(N) @ _prefilter_mat(N)


_NC_CACHE = {}


def _strip_redundant_self_waits(nc):
    """Drop sem waits that are trivially satisfied by same-engine program order.

    Tile's per-proc wait emission is not transitively minimal: a PE matmul can
    end up waiting on the PE's own semaphore (already guaranteed by in-order
    engine execution) in addition to a cross-engine wait, and the MM ISA
    struct only has one sync-wait slot (walrus: "Too many sync wait
    commands"). A wait on sem S is redundant for instruction I on engine E iff
    S is only ever updated by E and the cumulative updates to S from E before
    I already reach the wait value.
    """
    import concourse.mybir as mybir

    for fn in nc.m.functions:
        for blk in fn.blocks:
            updaters = {}  # sem id -> set of engines updating it (block-wide)
            for i in blk.instructions:
                si = i.sync_info
                if si is None:
                    continue
                for u in si.on_update or []:
                    updaters.setdefault(u.id, set()).add(i.engine)
            seen = {}  # (engine, sem id) -> cumulative update count so far
            for i in blk.instructions:
                si = i.sync_info
                if si is None:
                    continue
                if si.on_wait:
                    kept = []
                    for w in si.on_wait:
                        if (
                            w.sync_type == "semaphore"
                            and w.wait_mode == "sem-ge-imm"
                            and updaters.get(w.id) == {i.engine}
                            and seen.get((i.engine, w.id), 0) >= w.wait_value
                        ):
                            continue  # implied by program order
                        kept.append(w)
                    if len(kept) != len(si.on_wait):
                        si.on_wait[:] = kept
                for u in si.on_update or []:
                    key = (i.engine, u.id)
                    seen[key] = seen.get(key, 0) + u.update_value
            # each engine ISA struct has a single sync-wait slot: offload
            # extra waits onto same-engine nops inserted just before
            new_insts = []
            nop_n = 0
            for i in blk.instructions:
                si = i.sync_info
                if si is not None and si.on_wait and len(si.on_wait) > 1:
                    extra = list(si.on_wait[:-1])
                    si.on_wait[:] = [si.on_wait[-1]]
                    for w in extra:
                        nop = mybir.InstNoOp(
                            name=f"I-waitnop-{blk.name}-{nop_n}", ins=[], outs=[]
                        )
                        nop_n += 1
                        nop.engine = i.engine
                        nop.sync_info = mybir.SyncInfo(on_wait=[w], on_update=[])
                        new_insts.append(nop)
                new_insts.append(i)
            if nop_n:
                blk.instructions[:] = new_insts


def _hoist_input_dmas(nc, n_hoist=14):
    """Move the first input DMAs ahead of the sync engine's entry barrier.

    The Tile/BSP prologue (entry EVSEM barrier + TENSOR_LOAD) delays the
    first dma_start by ~7us. The leading input DMAs have no waits (inputs
    are resident at NEFF start, dst tiles untouched), so issuing them first
    starts the HBM reads during the prologue.
    """
    import concourse.mybir as mybir

    blocks = nc.m.functions[0].blocks
    body = blocks[1]
    dmas = []
    for i in body.instructions:
        if type(i).__name__ == "InstDMACopy" and i.engine == mybir.EngineType.SP:
            si = i.sync_info
            if si is not None and si.on_wait:
                break  # stop at the first gated DMA
            dmas.append(i)
            if len(dmas) >= n_hoist:
                break
    if not dmas:
        return
    dset = set(id(x) for x in dmas)
    body.instructions[:] = [i for i in body.instructions if id(i) not in dset]
    # insert into the prologue block after the leading InstCall, ahead of
    # the entry barrier: the sync engine starts immediately, so these DMAs
    # issue at t~0 while the other engines are still loading their code
    pro = blocks[0].instructions
    pos = 1 if pro and type(pro[0]).__name__ == "InstCall" else 0
    pro[:] = pro[:pos] + dmas + pro[pos:]


def build_nc():
    import concourse.bass as bass
    import concourse.mybir as mybir
    from concourse.tile import TileContext

    bf16 = mybir.dt.bfloat16
    f32 = mybir.dt.float32


    nc = bass.Bass(enable_partition_id=False)
    vol_ext = nc.declare_dram_parameter("vol", [3, ZWIN, 96 * 128], bf16, isOutput=False)
    mzt_ext = nc.declare_dram_parameter("mzt", [3, 128, 48], bf16, isOutput=False)
    mt_ext = nc.declare_dram_parameter("mt", [128, 192], bf16, isOutput=False)
    out_ext = nc.declare_dram_parameter("out", [3, 128, 72 * 192], bf16, isOutput=True)

    with TileContext(nc) as tc:
        with (
            tc.tile_pool(name="consts", bufs=1) as consts,
            tc.tile_pool(name="vols", bufs=1) as vols_pool,
            tc.tile_pool(name="l1", bufs=2) as l1_pool,
            tc.tile_pool(name="l2", bufs=2) as l2_pool,
            tc.tile_pool(name="stage", bufs=6) as stage_pool,
            tc.tile_pool(name="ps", bufs=4, space="PSUM") as ps_pool,
        ):
            mt = consts.tile([128, 192], bf16)
            nc.sync.dma_start(out=mt[:], in_=mt_ext[:])
            mzt_all = consts.tile([128, 3, 48], bf16)
            nc.sync.dma_start(out=mzt_all[:], in_=mzt_ext[:].transpose([1, 0, 2]))

            # PSUM evacuation alternated over the two PSUM-capable engines
            def _mk_copy(nc):
                state = [0]

                def cp(dst, src):
                    if state[0] % 2 == 0:
                        nc.scalar.copy(dst, src)
                    else:
                        nc.vector.tensor_copy(dst, src)
                    state[0] += 1

                return cp

            copy = _mk_copy(nc)

            # PE pacing: junk matmuls into the current PSUM tile's unused
            # padding columns (never read).  They keep the PE executing at a
            # production rate matched to the DVE+Act evacuation drain rate,
            # so it never idles (idle >~100ns drops the DVFS p-state from
            # 2.4 to 1.2 GHz).  BC phases with an interleaved stage A are
            # already diluted by its LDW-bound matmuls and get ~58ns of
            # pacing per tile; task 2's BC phase has no stage A to absorb
            # and gets ~130ns.
            def junk_mm(out):
                nc.tensor.matmul(
                    out, lhsT=mt[:, 0:128], rhs=mt[:, 0 : out.free_size()],
                    start=True, stop=True,
                )

            # ONE shared vol tile: rows 0:64 are re-DMAed per task (WAR
            # deps order each chunk behind the previous task's stage-A
            # reads of those columns); rows 64:128 are zeroed exactly once.
            vol = vols_pool.tile([128, 96 * 128], bf16, name="vol")
            # task 0's chunks alternate between the sync and scalar DGEs
            # (both idle during the prologue; a single queue moves only
            # ~50 GB/s so one engine alone gates stage A's start)
            bounds = [0, 512, 1536, 3072, 5632, 8704, 12288]

            def load_vol(t):
                for ch in range(6):
                    eng = nc.scalar if (t == 0 and ch % 2 == 1) else nc.sync
                    eng.dma_start(
                        out=vol[0:ZWIN, bounds[ch] : bounds[ch + 1]],
                        in_=vol_ext[t, :, bounds[ch] : bounds[ch + 1]],
                    )

            L1s = [
                l1_pool.tile([128, 48, 128], bf16, name="L1a"),
                l1_pool.tile([128, 48, 128], bf16, name="L1b"),
            ]
            # junk-row zeroing BEFORE the input DMA: the region tracker is
            # not partition-aware, so whichever comes second waits on the
            # first -- the memsets can start the moment the engines boot,
            # the DMA can't.  12 pieces alternating DVE/gpsimd so the
            # per-piece WAR chain clears left to right just ahead of both
            # the DMA chunks and stage A.
            for g in range(12):
                eng = nc.vector if g % 2 == 0 else nc.gpsimd
                eng.memset(vol[ZWIN:128, g * 1024 : (g + 1) * 1024], 0.0)
            nc.gpsimd.memset(L1s[0][:, :, 96:128], 0.0)
            nc.gpsimd.memset(L1s[1][:, :, 96:128], 0.0)
            load_vol(0)

            def emit_pa(tt, g):
                # one 2-bank PSUM tile: 16 x-slices of stage A for task tt
                L1 = L1s[tt % 2]
                pa = ps_pool.tile(
                    [128, 2, 48, 8], f32, name="pa", tag="ps",
                    padded_shape=[128, 2, 64, 8],
                )
                for b in range(2):
                    for j in range(8):
                        x = g * 16 + b * 8 + j
                        nc.tensor.matmul(
                            pa[:, b, :, j],
                            lhsT=vol[:, x * 128 : (x + 1) * 128],
                            rhs=mzt_all[:, tt, :],
                            start=True,
                            stop=True,
                        )
                copy(
                    L1[:, :, g * 16 : (g + 1) * 16].rearrange(
                        "p z (b j) -> p b z j", b=2
                    ),
                    pa[:, :, :, :],
                )

            def emit_pb(t, zz, L1, L2, big):
                pb = ps_pool.tile([128, 2, 2, 256], f32, name="pb", tag="ps")
                for b in range(2):
                    for jj in range(2):
                        zp = zz * 4 + b * 2 + jj
                        nc.tensor.matmul(
                            pb[:, b, jj, 0:192],
                            lhsT=L1[:, zp, :],
                            rhs=mt[:],
                            start=True,
                            stop=True,
                        )
                if big:
                    junk_mm(pb[:, 0, :, 192:256])
                    junk_mm(pb[:, 1, :, 192:240])
                else:
                    junk_mm(pb[:, 0, :, 192:240])
                copy(
                    L2[:, zz * 4 : zz * 4 + 4, :].rearrange(
                        "p (b j) y -> p b j y", b=2
                    ),
                    pb[:, :, :, 0:192],
                )

            def make_emit_c(t, L2f, big):
                # pc groups: 18 per task, 4 chunks (2 PSUM banks) each.
                gper = 3 if t < 2 else 2
                stage_tiles = {}

                def emit_c_group(g):
                    q, gq = divmod(g, gper)
                    if gq == 0:
                        stage_tiles[q] = stage_pool.tile(
                            [128, 4 * gper, 192], bf16, name="stage"
                        )
                    stage = stage_tiles[q]
                    pc = ps_pool.tile([128, 4, 256], f32, name="pc", tag="ps")
                    for j in range(4):
                        ch = g * 4 + j
                        nc.tensor.matmul(
                            pc[:, j, 0:192],
                            lhsT=L2f[:, ch * 128 : (ch + 1) * 128],
                            rhs=mt[:],
                            start=True,
                            stop=True,
                        )
                    if big:
                        junk_mm(pc[:, 0:2, 192:256])
                        junk_mm(pc[:, 2:4, 192:240])
                    else:
                        junk_mm(pc[:, 0:2, 192:240])
                    copy(stage[:, gq * 4 : gq * 4 + 4, :], pc[:, :, 0:192])
                    if gq == gper - 1:
                        cols = 4 * gper * 192
                        nc.sync.dma_start(
                            out=out_ext[t, :, q * cols : (q + 1) * cols],
                            in_=stage[:],
                        )

                return emit_c_group

            # ---- task 0 stage A, solo (LDW-bound: already below the
            # drain rate, no pacing nops needed) ----
            for g in range(6):
                emit_pa(0, g)

            # ---- per task: B and C interleaved, plus the NEXT task's
            # stage A spread through the phase (its low PSUM production
            # rate dilutes B/C's toward the evacuation drain rate) ----
            for t in range(3):
                big = t == 2
                if t < 2:
                    load_vol(t + 1)  # WAR-ordered behind task t's A reads
                L1 = L1s[t % 2]
                L2 = l2_pool.tile([128, 48, 192], bf16)
                L2f = L2[:].rearrange("p a b -> p (a b)")  # (128, 9216)
                emit_c = make_emit_c(t, L2f, big)

                bc = []
                g_next = 0
                for zz in range(12):
                    bc.append(("b", zz))
                    rows_done = (zz + 1) * 4 * 192
                    while g_next < 18 and (g_next * 4 + 4) * 128 <= rows_done:
                        bc.append(("c", g_next))
                        g_next += 1
                while g_next < 18:
                    bc.append(("c", g_next))
                    g_next += 1

                ai = 0
                for i, (kind, idx) in enumerate(bc):
                    if kind == "b":
                        emit_pb(t, idx, L1, L2, big)
                    else:
                        emit_c(idx)
                    if t < 2 and ai < 6 and i in (3, 8, 13, 17, 21, 25):
                        emit_pa(t + 1, ai)
                        ai += 1
    _strip_redundant_self_waits(nc)
    _hoist_input_dmas(nc)
    return nc


def _task_map(core, t):
    gt = NTASK_PER_CORE * core + t
    bc, s = divmod(gt, 4)
    b, c = divmod(bc, 3)
    return b, c, s


def make_in_maps(volume, M):
    mt_b = np.zeros((128, 192), dtype=BF16)  # K zero-padded to 128 rows
    mt_b[:96] = np.ascontiguousarray(M.T).astype(BF16)
    in_maps = []
    for core in range(NCORES):
        vols = np.zeros((3, ZWIN, 96, 128), dtype=BF16)
        mzts = np.zeros((3, 128, 48), dtype=BF16)
        for t in range(NTASK_PER_CORE):
            b, c, s = _task_map(core, t)
            w0 = WINS[s]
            vt = np.transpose(volume[b, c], (0, 2, 1))  # (z, x, y)
            vols[t, :, :, :96] = vt[w0 : w0 + ZWIN].astype(BF16)
            mzts[t, :ZWIN] = np.ascontiguousarray(
                M[s * ZSLICE : (s + 1) * ZSLICE, w0 : w0 + ZWIN].T
            ).astype(BF16)
        in_maps.append(
            {"vol": vols.reshape(3, ZWIN, 96 * 128), "mzt": mzts, "mt": mt_b}
        )
    return in_maps


def gather_out(results):
    out = np.zeros((2, 3, 192, 192, 192), dtype=np.float32)
    # flat (z'l, y') index f = ch*128 + p ; o[t, p, ch, :] = out[.., z'l, y', :]
    f = np.arange(48 * 192)
    ch, p = np.divmod(f, 128)
    zl, yp = np.divmod(f, 192)
    for core in range(NCORES):
        o = np.asarray(results[core]["out"], dtype=np.float32).reshape(
            3, 128, 72, 192
        )
        for t in range(NTASK_PER_CORE):
            b, c, s = _task_map(core, t)
            out[b, c, s * ZSLICE + zl, yp, :] = o[t, p, ch, :]
    return out


def run(volume, trace=False):
    """Returns (output, exec_time_ns_or_None)."""
    import concourse.bass_utils as bu
    from concourse.bass_utils import run_bass_kernel_spmd

    if trace:
        # avoid the S3 artifact upload in the axon trace path
        bu.upload_artifacts = lambda tmpdir: str(tmpdir)

    volume = np.asarray(volume, dtype=np.float32)
    M = build_M()
    in_maps = make_in_maps(volume, M)
    if "nc" not in _NC_CACHE:
        _NC_CACHE["nc"] = build_nc()
    nc = _NC_CACHE["nc"]
    res = run_bass_kernel_spmd(
        nc, in_maps, core_ids=list(range(NCORES)), trace=trace
    )
    out = gather_out(res.results)
    return out, getattr(res, "exec_time_ns", None)


def kernel(volume):
    out, _ = run(volume, trace=False)
    return out
